# revision 9
# baseline (speedup 1.0000x reference)
"""Bhattacharyya coefficient kernel for Trainium2 (8 NeuronCores, SPMD).

out[n,0,i,j] = (1/k^2) * sum_{c,p,q} w[c] * sqrt(x[n,c,i+p,j+q] * z[n,c,p,q])

Data-parallel over batch: 2 samples per core. Per sample:
  1. ACT: sx = sqrt(x) (bf16), szw = w/k^2 * sqrt(z) (bf16).
  2. TensorE: plane[t, y] = sum_c szw[c, t] * sx[c, y] for the 64 taps
     t = 8p+q and all 63*63 image pixels y (K=256 in two 128-chunks
     accumulated in PSUM, M=64 taps, N in 8 blocks of <=512).
  3. Evict PSUM -> SBUF (bf16, DVE), dump plane to a DRAM scratch (SWDGE).
  4. Gather from DRAM with per-tap shifted offsets (flat DRAM AP):
     A[t, u] = plane[t, u + 63*(t>>3) + (t&7)], so the tap-sum becomes a
     pure partition reduction.
  5. TensorE ones-matmul: o[u] = sum_t A[t, u]; evict; out[i,j] = o[63i+j].

Input x DMAs are split into ~1MB pieces and issued up-front on the Sync
HWDGE ring (kept free of dependent waits) so HBM streaming overlaps the
sqrt/matmul pipeline from the start.
"""

import numpy as np

import concourse.bacc as bacc
import concourse.bass as bass
import concourse.mybir as mybir
from concourse import tile
from concourse.bass_utils import run_bass_kernel_spmd

N, C, KS, MS = 16, 256, 8, 63
MO = MS - KS + 1            # 56
F = MS * MS                 # 3969
L = (MO - 1) * MS + MO + 2  # 3522 (even; last needed flat index is 63*55+55)
NCORES = 8
SPC = N // NCORES           # samples per core
BLK = 512
NBLK = (F + BLK - 1) // BLK   # 8
NBLK2 = (L + BLK - 1) // BLK  # 7
HALF = 4 * BLK                # y-piece size for x staging (2048)
AF = mybir.ActivationFunctionType
f32 = mybir.dt.float32
bf16 = mybir.dt.bfloat16

_CACHE = {}


def _build():
    nc = bacc.Bacc("TRN2", target_bir_lowering=False, debug=False)
    z_in = nc.declare_dram_parameter("z", [SPC, C, KS, KS], f32, isOutput=False)
    x_in = nc.declare_dram_parameter("x", [SPC, C, MS, MS], f32, isOutput=False)
    w_in = nc.declare_dram_parameter("w", [C], f32, isOutput=False)
    out = nc.declare_dram_parameter("out", [SPC, 1, MO, MO], f32, isOutput=True)
    FP = F + 8  # padded row pitch in the DRAM scratch (AP slack)
    pl_dram = nc.dram_tensor("pl_scratch", [SPC, 64, FP], bf16)

    xflat = x_in.rearrange("s (k c) h w -> s k c (h w)", c=128)  # [SPC, 2, 128, F]

    with tile.TileContext(nc) as tc:
        with (
            tc.tile_pool(name="xstage", bufs=5) as xstage,
            tc.tile_pool(name="sxq", bufs=2) as sxq,
            tc.tile_pool(name="zpool", bufs=2) as zpool,
            tc.tile_pool(name="plane", bufs=2) as plane,
            tc.tile_pool(name="gath", bufs=2) as gath,
            tc.tile_pool(name="opool", bufs=2) as opool,
            tc.tile_pool(name="psum", bufs=4, space="PSUM") as psum,
            tc.tile_pool(name="psum2", bufs=4, space="PSUM") as psum2,
        ):
            # all-ones [128, 1] bf16 for the tap-reduction matmul
            ones = opool.tile([128, 1], bf16, name="ones")
            nc.gpsimd.memset(ones[:], 1.0)

            # ---- input DMAs up-front, all on the Sync HWDGE ring ----
            xst = {}
            for s in range(SPC):
                for k in range(2):
                    for h in range(2):
                        nh = min(HALF, F - h * HALF)
                        t = xstage.tile([128, HALF], f32, tag="xst",
                                        name=f"xst{s}{k}{h}")
                        nc.sync.dma_start(
                            t[:, :nh], xflat[s, k, :, h * HALF : h * HALF + nh]
                        )
                        xst[(s, k, h)] = t
            zts = []
            for s in range(SPC):
                zt = zpool.tile([128, 2, KS * KS], f32, tag="zt", name=f"zt{s}")
                nc.sync.dma_start(
                    zt[:], z_in[s].rearrange("(k c) p q -> c k (p q)", c=128)
                )
                zts.append(zt)
            wt = zpool.tile([128, 2], f32, name="wt")
            nc.sync.dma_start(wt[:], w_in.rearrange("(k c) -> c k", c=128))
            w64 = zpool.tile([128, 2], f32, name="w64")
            nc.scalar.mul(w64[:], wt[:], 1.0 / (KS * KS))

            for s in range(SPC):
                # ---- z path: szw[c, k, t] = w[c]/64 * sqrt(z[c, t]) ----
                zsq = zpool.tile([128, 2, KS * KS], f32, tag="zsq", name=f"zsq{s}")
                szw = zpool.tile([128, 2, KS * KS], bf16, tag="szw", name=f"szw{s}")
                for k in range(2):
                    nc.scalar.activation(zsq[:, k, :], zts[s][:, k, :], AF.Sqrt)
                    nc.vector.tensor_scalar_mul(
                        szw[:, k, :], zsq[:, k, :], w64[:, k : k + 1]
                    )

                # ---- x path: sx[c, k, y] = sqrt(x[c_global, y]) (bf16) ----
                sx = sxq.tile([128, 2, F], bf16, tag="sx", name=f"sx{s}")
                for k in range(2):
                    for h in range(2):
                        nh = min(HALF, F - h * HALF)
                        nc.scalar.activation(
                            sx[:, k, h * HALF : h * HALF + nh],
                            xst[(s, k, h)][:, :nh],
                            AF.Sqrt,
                        )

                # ---- matmuls: plane[t, y] = sum_c szw[c, t] sx[c, y] ----
                pl = plane.tile([64, F], bf16, tag="pl", name=f"pl{s}")
                for h in range(2):
                    for b in range(4 * h, min(4 * h + 4, NBLK)):
                        nb = min(BLK, F - b * BLK)
                        ps = psum.tile([64, BLK], f32, tag="ps", name=f"ps_{s}_{b}")
                        for k in range(2):
                            nc.tensor.matmul(
                                ps[:, :nb],
                                szw[:, k, :],
                                sx[:, k, b * BLK : b * BLK + nb],
                                start=(k == 0),
                                stop=(k == 1),
                            )
                        # evict (cast bf16) on DVE
                        nc.vector.tensor_copy(
                            pl[:, b * BLK : b * BLK + nb], ps[:, :nb]
                        )
                    # dump this half of the plane to DRAM scratch (SWDGE)
                    lo = 4 * h * BLK
                    hi = min(4 * h * BLK + 4 * BLK, F)
                    nc.gpsimd.dma_start(pl_dram[s, :, lo:hi], pl[:, lo:hi])

                # ---- gather with per-tap shift: A[t, u] = plane[t, u+off(t)]
                a2 = gath.tile([64, L], bf16, tag="a2", name=f"a2_{s}")
                src = bass.AP(
                    pl_dram[:].tensor,
                    s * 64 * FP,
                    [[8 * FP + MS, 8], [FP + 1, 8], [1, L]],
                )
                nc.gpsimd.dma_start(a2[:], src)

                # ---- tap reduction: o[u] = sum_t A[t, u] ----
                obuf = opool.tile([1, 3584], f32, tag=f"ob{s}", name=f"obuf{s}")
                for b in range(NBLK2):
                    nb = min(BLK, L - b * BLK)
                    ps2 = psum2.tile([1, BLK], f32, tag="ps2", name=f"ps2_{s}_{b}")
                    nc.tensor.matmul(
                        ps2[:, :nb],
                        ones[0:64, :],
                        a2[:, b * BLK : b * BLK + nb],
                        start=True,
                        stop=True,
                    )
                    nc.vector.tensor_copy(obuf[0:1, b * BLK : b * BLK + nb], ps2[:, :nb])

                # ---- extract valid rows: out[i, j] = o[63 i + j] ----
                osrc = obuf[0:1, 0 : MO * MS].rearrange("p (i j) -> p i j", i=MO)[
                    :, :, 0:MO
                ]
                nc.gpsimd.dma_start(out[s, 0].unsqueeze(0), osrc)

    nc.compile()
    return nc


def _get_nc():
    if "nc" not in _CACHE:
        _CACHE["nc"] = _build()
    return _CACHE["nc"]


def _run(z, x, weights, **runkw):
    z = np.ascontiguousarray(np.asarray(z), dtype=np.float32)
    x = np.ascontiguousarray(np.asarray(x), dtype=np.float32)
    w = np.ascontiguousarray(np.asarray(weights), dtype=np.float32).reshape(C)
    in_maps = []
    for i in range(NCORES):
        lo, hi = i * SPC, (i + 1) * SPC
        in_maps.append({"z": z[lo:hi], "x": x[lo:hi], "w": w})
    nc = _get_nc()
    res = run_bass_kernel_spmd(nc, in_maps, core_ids=list(range(NCORES)), **runkw)
    full = np.concatenate([res.results[i]["out"] for i in range(NCORES)], axis=0)
    return full, res


def kernel(z, x, weights):
    full, _ = _run(z, x, weights)
    return full


# revision 10
# speedup vs baseline: 1.0992x; 1.0992x over previous
"""Bhattacharyya coefficient kernel for Trainium2 (8 NeuronCores, SPMD).

out[n,0,i,j] = (1/k^2) * sum_{c,p,q} w[c] * sqrt(x[n,c,i+p,j+q] * z[n,c,p,q])

Data-parallel over batch: 2 samples per core. Per sample:
  1. ACT: sx = sqrt(x) (bf16), szw = w/k^2 * sqrt(z) (bf16).
  2. TensorE: plane[t, y] = sum_c szw[c, t] * sx[c, y] for the 64 taps
     t = 8p+q and all 63*63 image pixels y (K=256 in two 128-chunks
     accumulated in PSUM, M=64 taps, N in 8 blocks of <=512).
  3. Evict PSUM -> SBUF (bf16, DVE), dump plane to a DRAM scratch (SWDGE).
  4. Gather from DRAM with per-tap shifted offsets (flat DRAM AP):
     A[t, u] = plane[t, u + 63*(t>>3) + (t&7)], so the tap-sum becomes a
     pure partition reduction.
  5. TensorE ones-matmul: o[u] = sum_t A[t, u]; evict; out[i,j] = o[63i+j].

Input x DMAs are split into ~1MB pieces and issued up-front on the Sync
HWDGE ring (kept free of dependent waits) so HBM streaming overlaps the
sqrt/matmul pipeline from the start.
"""

import numpy as np

import concourse.bacc as bacc
import concourse.bass as bass
import concourse.mybir as mybir
from concourse import tile
from concourse.bass_utils import run_bass_kernel_spmd

N, C, KS, MS = 16, 256, 8, 63
MO = MS - KS + 1            # 56
F = MS * MS                 # 3969
L = (MO - 1) * MS + MO + 2  # 3522 (even; last needed flat index is 63*55+55)
NCORES = 8
SPC = N // NCORES           # samples per core
BLK = 512
NBLK = (F + BLK - 1) // BLK   # 8
NBLK2 = (L + BLK - 1) // BLK  # 7
HALF = 4 * BLK                # y-piece size for x staging (2048)
AF = mybir.ActivationFunctionType
f32 = mybir.dt.float32
bf16 = mybir.dt.bfloat16

_CACHE = {}


def _build():
    nc = bacc.Bacc("TRN2", target_bir_lowering=False, debug=False)
    z_in = nc.declare_dram_parameter("z", [SPC, C, KS, KS], f32, isOutput=False)
    x_in = nc.declare_dram_parameter("x", [SPC, C, MS, MS], f32, isOutput=False)
    w_in = nc.declare_dram_parameter("w", [C], f32, isOutput=False)
    out = nc.declare_dram_parameter("out", [SPC, 1, MO, MO], f32, isOutput=True)
    FP = F + 8  # padded row pitch in the DRAM scratch (AP slack)
    pl_dram = nc.dram_tensor("pl_scratch", [SPC, 64, FP], bf16)

    xflat = x_in.rearrange("s (k c) h w -> s k c (h w)", c=128)  # [SPC, 2, 128, F]

    with tile.TileContext(nc) as tc:
        with (
            tc.tile_pool(name="xstage", bufs=5) as xstage,
            tc.tile_pool(name="sxq", bufs=8) as sxq,
            tc.tile_pool(name="zpool", bufs=2) as zpool,
            tc.tile_pool(name="plane", bufs=4) as plane,
            tc.tile_pool(name="gath", bufs=2) as gath,
            tc.tile_pool(name="opool", bufs=2) as opool,
            tc.tile_pool(name="psum", bufs=4, space="PSUM") as psum,
            tc.tile_pool(name="psum2", bufs=4, space="PSUM") as psum2,
        ):
            # all-ones [128, 1] bf16 for the tap-reduction matmul
            ones = opool.tile([128, 1], bf16, name="ones")
            nc.gpsimd.memset(ones[:], 1.0)

            # ---- input DMAs up-front, all on the Sync HWDGE ring ----
            xst = {}
            for s in range(SPC):
                for k in range(2):
                    for h in range(2):
                        nh = min(HALF, F - h * HALF)
                        t = xstage.tile([128, HALF], f32, tag="xst",
                                        name=f"xst{s}{k}{h}")
                        nc.sync.dma_start(
                            t[:, :nh], xflat[s, k, :, h * HALF : h * HALF + nh]
                        )
                        xst[(s, k, h)] = t
            zts = []
            for s in range(SPC):
                zt = zpool.tile([128, 2, KS * KS], f32, tag="zt", name=f"zt{s}")
                nc.sync.dma_start(
                    zt[:], z_in[s].rearrange("(k c) p q -> c k (p q)", c=128)
                )
                zts.append(zt)
            wt = zpool.tile([128, 2], f32, name="wt")
            nc.sync.dma_start(wt[:], w_in.rearrange("(k c) -> c k", c=128))
            w64 = zpool.tile([128, 2], f32, name="w64")
            nc.scalar.mul(w64[:], wt[:], 1.0 / (KS * KS))

            for s in range(SPC):
                # ---- z path: szw[c, k, t] = w[c]/64 * sqrt(z[c, t]) ----
                zsq = zpool.tile([128, 2, KS * KS], f32, tag="zsq", name=f"zsq{s}")
                szw = zpool.tile([128, 2, KS * KS], bf16, tag="szw", name=f"szw{s}")
                for k in range(2):
                    nc.scalar.activation(zsq[:, k, :], zts[s][:, k, :], AF.Sqrt)
                    nc.vector.tensor_scalar_mul(
                        szw[:, k, :], zsq[:, k, :], w64[:, k : k + 1]
                    )

                # ---- x path: per-piece sqrt (bf16), separate tiles so
                # matmuls only depend on the pieces they read ----
                sxp = {}
                for k in range(2):
                    for h in range(2):
                        nh = min(HALF, F - h * HALF)
                        t = sxq.tile([128, HALF], bf16, tag="sxp",
                                     name=f"sxp{s}{k}{h}")
                        nc.scalar.activation(
                            t[:, :nh], xst[(s, k, h)][:, :nh], AF.Sqrt
                        )
                        sxp[(k, h)] = t

                # ---- matmuls: plane[t, y] = sum_c szw[c, t] sx[c, y] ----
                plh = {}
                for h in range(2):
                    nhalf = min(HALF, F - h * HALF)
                    pl = plane.tile([64, HALF], bf16, tag="pl", name=f"pl{s}{h}")
                    plh[h] = pl
                    pss = [
                        psum.tile([64, BLK], f32, tag="ps", name=f"ps_{s}_{4*h+j}")
                        for j in range(4)
                    ]
                    for k in range(2):
                        for j in range(4):
                            b = 4 * h + j
                            nb = min(BLK, F - b * BLK)
                            nc.tensor.matmul(
                                pss[j][:, :nb],
                                szw[:, k, :],
                                sxp[(k, h)][:, j * BLK : j * BLK + nb],
                                start=(k == 0),
                                stop=(k == 1),
                            )
                    for j in range(4):
                        b = 4 * h + j
                        nb = min(BLK, F - b * BLK)
                        # evict (cast bf16) on DVE
                        nc.vector.tensor_copy(
                            pl[:, j * BLK : j * BLK + nb], pss[j][:, :nb]
                        )
                    # dump this half of the plane to DRAM scratch (SWDGE)
                    nc.gpsimd.dma_start(
                        pl_dram[s, :, h * HALF : h * HALF + nhalf], pl[:, :nhalf]
                    )

                # ---- gather with per-tap shift: A[t, u] = plane[t, u+off(t)]
                a2 = gath.tile([64, L], bf16, tag="a2", name=f"a2_{s}")
                src = bass.AP(
                    pl_dram[:].tensor,
                    s * 64 * FP,
                    [[8 * FP + MS, 8], [FP + 1, 8], [1, L]],
                )
                nc.gpsimd.dma_start(a2[:], src)

                # ---- tap reduction: o[u] = sum_t A[t, u] ----
                obuf = opool.tile([1, 3584], f32, tag=f"ob{s}", name=f"obuf{s}")
                for b in range(NBLK2):
                    nb = min(BLK, L - b * BLK)
                    ps2 = psum2.tile([1, BLK], f32, tag="ps2", name=f"ps2_{s}_{b}")
                    nc.tensor.matmul(
                        ps2[:, :nb],
                        ones[0:64, :],
                        a2[:, b * BLK : b * BLK + nb],
                        start=True,
                        stop=True,
                    )
                    if b % 2 == 0:
                        nc.scalar.copy(
                            obuf[0:1, b * BLK : b * BLK + nb], ps2[:, :nb]
                        )
                    else:
                        nc.vector.tensor_copy(
                            obuf[0:1, b * BLK : b * BLK + nb], ps2[:, :nb]
                        )

                # ---- extract valid rows: out[i, j] = o[63 i + j] ----
                osrc = obuf[0:1, 0 : MO * MS].rearrange("p (i j) -> p i j", i=MO)[
                    :, :, 0:MO
                ]
                nc.gpsimd.dma_start(out[s, 0].unsqueeze(0), osrc)

    nc.compile()
    return nc


def _get_nc():
    if "nc" not in _CACHE:
        _CACHE["nc"] = _build()
    return _CACHE["nc"]


def _run(z, x, weights, **runkw):
    z = np.ascontiguousarray(np.asarray(z), dtype=np.float32)
    x = np.ascontiguousarray(np.asarray(x), dtype=np.float32)
    w = np.ascontiguousarray(np.asarray(weights), dtype=np.float32).reshape(C)
    in_maps = []
    for i in range(NCORES):
        lo, hi = i * SPC, (i + 1) * SPC
        in_maps.append({"z": z[lo:hi], "x": x[lo:hi], "w": w})
    nc = _get_nc()
    res = run_bass_kernel_spmd(nc, in_maps, core_ids=list(range(NCORES)), **runkw)
    full = np.concatenate([res.results[i]["out"] for i in range(NCORES)], axis=0)
    return full, res


def kernel(z, x, weights):
    full, _ = _run(z, x, weights)
    return full


# revision 11
# speedup vs baseline: 1.1263x; 1.0246x over previous
"""Bhattacharyya coefficient kernel for Trainium2 (8 NeuronCores, SPMD).

out[n,0,i,j] = (1/k^2) * sum_{c,p,q} w[c] * sqrt(x[n,c,i+p,j+q] * z[n,c,p,q])

Data-parallel over batch: 2 samples per core. Per sample:
  1. ACT: sx = sqrt(x) (bf16), szw = w/k^2 * sqrt(z) (bf16).
  2. TensorE: plane[t, y] = sum_c szw[c, t] * sx[c, y] for the 64 taps
     t = 8p+q and all 63*63 image pixels y (K=256 in two 128-chunks
     accumulated in PSUM, M=64 taps, N in 8 blocks of <=512).
  3. Evict PSUM -> SBUF (bf16, DVE), dump plane to a DRAM scratch (SWDGE).
  4. Gather from DRAM with per-tap shifted offsets (flat DRAM AP):
     A[t, u] = plane[t, u + 63*(t>>3) + (t&7)], so the tap-sum becomes a
     pure partition reduction.
  5. TensorE ones-matmul: o[u] = sum_t A[t, u]; evict; out[i,j] = o[63i+j].

Input x DMAs are split into ~1MB pieces and issued up-front on the Sync
HWDGE ring (kept free of dependent waits) so HBM streaming overlaps the
sqrt/matmul pipeline from the start.
"""

import numpy as np

import concourse.bacc as bacc
import concourse.bass as bass
import concourse.mybir as mybir
from concourse import tile
from concourse.bass_utils import run_bass_kernel_spmd

N, C, KS, MS = 16, 256, 8, 63
MO = MS - KS + 1            # 56
F = MS * MS                 # 3969
L = (MO - 1) * MS + MO + 2  # 3522 (even; last needed flat index is 63*55+55)
NCORES = 8
SPC = N // NCORES           # samples per core
BLK = 512
NBLK = (F + BLK - 1) // BLK   # 8
NBLK2 = (L + BLK - 1) // BLK  # 7
HALF = 4 * BLK                # y-piece size for x staging (2048)
AF = mybir.ActivationFunctionType
f32 = mybir.dt.float32
bf16 = mybir.dt.bfloat16

_CACHE = {}


def _build():
    nc = bacc.Bacc("TRN2", target_bir_lowering=False, debug=False)
    z_in = nc.declare_dram_parameter("z", [SPC, C, KS, KS], f32, isOutput=False)
    x_in = nc.declare_dram_parameter("x", [SPC, C, MS, MS], f32, isOutput=False)
    w_in = nc.declare_dram_parameter("w", [C], f32, isOutput=False)
    out = nc.declare_dram_parameter("out", [SPC, 1, MO, MO], f32, isOutput=True)
    FP = F + 8  # padded row pitch in the DRAM scratch (AP slack)
    pl_dram = nc.dram_tensor("pl_scratch", [SPC, 64, FP], bf16)

    xflat = x_in.rearrange("s (k c) h w -> s k c (h w)", c=128)  # [SPC, 2, 128, F]

    with tile.TileContext(nc) as tc:
        with (
            tc.tile_pool(name="xstage", bufs=5) as xstage,
            tc.tile_pool(name="sxq", bufs=8) as sxq,
            tc.tile_pool(name="zpool", bufs=2) as zpool,
            tc.tile_pool(name="plane", bufs=4) as plane,
            tc.tile_pool(name="gath", bufs=2) as gath,
            tc.tile_pool(name="opool", bufs=2) as opool,
            tc.tile_pool(name="psum", bufs=4, space="PSUM") as psum,
            tc.tile_pool(name="psum2", bufs=4, space="PSUM") as psum2,
        ):
            # all-ones [128, 1] bf16 for the tap-reduction matmul
            ones = opool.tile([128, 1], bf16, name="ones")
            nc.gpsimd.memset(ones[:], 1.0)

            # ---- input DMAs up-front, all on the Sync HWDGE ring ----
            # (small z/w loads first so szw never gates the matmuls)
            wt = zpool.tile([128, 2], f32, name="wt")
            nc.sync.dma_start(wt[:], w_in.rearrange("(k c) -> c k", c=128))
            zts = []
            for s in range(SPC):
                zt = zpool.tile([128, 2, KS * KS], f32, tag="zt", name=f"zt{s}")
                nc.sync.dma_start(
                    zt[:], z_in[s].rearrange("(k c) p q -> c k (p q)", c=128)
                )
                zts.append(zt)
            w64 = zpool.tile([128, 2], f32, name="w64")
            nc.scalar.mul(w64[:], wt[:], 1.0 / (KS * KS))
            xst = {}
            for s in range(SPC):
                for h in range(2):
                    for k in range(2):
                        nh = min(HALF, F - h * HALF)
                        t = xstage.tile([128, HALF], f32, tag="xst",
                                        name=f"xst{s}{k}{h}")
                        nc.sync.dma_start(
                            t[:, :nh], xflat[s, k, :, h * HALF : h * HALF + nh]
                        )
                        xst[(s, k, h)] = t

            for s in range(SPC):
                # ---- z path: szw[c, k, t] = w[c]/64 * sqrt(z[c, t]) ----
                zsq = zpool.tile([128, 2, KS * KS], f32, tag="zsq", name=f"zsq{s}")
                szw = zpool.tile([128, 2, KS * KS], bf16, tag="szw", name=f"szw{s}")
                for k in range(2):
                    nc.scalar.activation(zsq[:, k, :], zts[s][:, k, :], AF.Sqrt)
                    nc.vector.tensor_scalar_mul(
                        szw[:, k, :], zsq[:, k, :], w64[:, k : k + 1]
                    )

                # ---- x path: per-piece sqrt (bf16), separate tiles so
                # matmuls only depend on the pieces they read ----
                sxp = {}
                for h in range(2):
                    for k in range(2):
                        nh = min(HALF, F - h * HALF)
                        t = sxq.tile([128, HALF], bf16, tag="sxp",
                                     name=f"sxp{s}{k}{h}")
                        nc.scalar.activation(
                            t[:, :nh], xst[(s, k, h)][:, :nh], AF.Sqrt
                        )
                        sxp[(k, h)] = t

                # ---- matmuls: plane[t, y] = sum_c szw[c, t] sx[c, y] ----
                plh = {}
                for h in range(2):
                    nhalf = min(HALF, F - h * HALF)
                    pl = plane.tile([64, HALF], bf16, tag="pl", name=f"pl{s}{h}")
                    plh[h] = pl
                    pss = [
                        psum.tile([64, BLK], f32, tag="ps", name=f"ps_{s}_{4*h+j}")
                        for j in range(4)
                    ]
                    for k in range(2):
                        for j in range(4):
                            b = 4 * h + j
                            nb = min(BLK, F - b * BLK)
                            nc.tensor.matmul(
                                pss[j][:, :nb],
                                szw[:, k, :],
                                sxp[(k, h)][:, j * BLK : j * BLK + nb],
                                start=(k == 0),
                                stop=(k == 1),
                            )
                    for j in range(4):
                        b = 4 * h + j
                        nb = min(BLK, F - b * BLK)
                        # evict (cast bf16) on DVE
                        nc.vector.tensor_copy(
                            pl[:, j * BLK : j * BLK + nb], pss[j][:, :nb]
                        )
                    # dump this half of the plane to DRAM scratch (SWDGE)
                    nc.gpsimd.dma_start(
                        pl_dram[s, :, h * HALF : h * HALF + nhalf], pl[:, :nhalf]
                    )

            for s in range(SPC):
                # ---- gather with per-tap shift: A[t, u] = plane[t, u+off(t)]
                a2 = gath.tile([64, L], bf16, tag="a2", name=f"a2_{s}")
                src = bass.AP(
                    pl_dram[:].tensor,
                    s * 64 * FP,
                    [[8 * FP + MS, 8], [FP + 1, 8], [1, L]],
                )
                nc.gpsimd.dma_start(a2[:], src)

                # ---- tap reduction: o[u] = sum_t A[t, u] ----
                obuf = opool.tile([1, 3584], f32, tag=f"ob{s}", name=f"obuf{s}")
                for b in range(NBLK2):
                    nb = min(BLK, L - b * BLK)
                    ps2 = psum2.tile([1, BLK], f32, tag="ps2", name=f"ps2_{s}_{b}")
                    nc.tensor.matmul(
                        ps2[:, :nb],
                        ones[0:64, :],
                        a2[:, b * BLK : b * BLK + nb],
                        start=True,
                        stop=True,
                    )
                    if b % 2 == 0:
                        nc.scalar.copy(
                            obuf[0:1, b * BLK : b * BLK + nb], ps2[:, :nb]
                        )
                    else:
                        nc.vector.tensor_copy(
                            obuf[0:1, b * BLK : b * BLK + nb], ps2[:, :nb]
                        )

                # ---- extract valid rows: out[i, j] = o[63 i + j] ----
                osrc = obuf[0:1, 0 : MO * MS].rearrange("p (i j) -> p i j", i=MO)[
                    :, :, 0:MO
                ]
                nc.gpsimd.dma_start(out[s, 0].unsqueeze(0), osrc)

    nc.compile()
    return nc


def _get_nc():
    if "nc" not in _CACHE:
        _CACHE["nc"] = _build()
    return _CACHE["nc"]


def _run(z, x, weights, **runkw):
    z = np.ascontiguousarray(np.asarray(z), dtype=np.float32)
    x = np.ascontiguousarray(np.asarray(x), dtype=np.float32)
    w = np.ascontiguousarray(np.asarray(weights), dtype=np.float32).reshape(C)
    in_maps = []
    for i in range(NCORES):
        lo, hi = i * SPC, (i + 1) * SPC
        in_maps.append({"z": z[lo:hi], "x": x[lo:hi], "w": w})
    nc = _get_nc()
    res = run_bass_kernel_spmd(nc, in_maps, core_ids=list(range(NCORES)), **runkw)
    full = np.concatenate([res.results[i]["out"] for i in range(NCORES)], axis=0)
    return full, res


def kernel(z, x, weights):
    full, _ = _run(z, x, weights)
    return full


# revision 12
# speedup vs baseline: 1.1681x; 1.0371x over previous
"""Bhattacharyya coefficient kernel for Trainium2 (8 NeuronCores, SPMD).

out[n,0,i,j] = (1/k^2) * sum_{c,p,q} w[c] * sqrt(x[n,c,i+p,j+q] * z[n,c,p,q])

Data-parallel over batch: 2 samples per core. Per sample:
  1. ACT: sx = sqrt(x) (bf16), szw = w/k^2 * sqrt(z) (bf16).
  2. TensorE: plane[t, y] = sum_c szw[c, t] * sx[c, y] for the 64 taps
     t = 8p+q and all 63*63 image pixels y (K=256 in two 128-chunks
     accumulated in PSUM, M=64 taps, N in 8 blocks of <=512).
  3. Evict PSUM -> SBUF (bf16, DVE), dump plane to a DRAM scratch (SWDGE).
  4. Gather from DRAM with per-tap shifted offsets (flat DRAM AP):
     A[t, u] = plane[t, u + 63*(t>>3) + (t&7)], so the tap-sum becomes a
     pure partition reduction.
  5. TensorE ones-matmul: o[u] = sum_t A[t, u]; evict; out[i,j] = o[63i+j].

Input x DMAs are split into ~1MB pieces and issued up-front on the Sync
HWDGE ring (kept free of dependent waits) so HBM streaming overlaps the
sqrt/matmul pipeline from the start.
"""

import numpy as np

import concourse.bacc as bacc
import concourse.bass as bass
import concourse.mybir as mybir
from concourse import tile
from concourse.bass_utils import run_bass_kernel_spmd

N, C, KS, MS = 16, 256, 8, 63
MO = MS - KS + 1            # 56
F = MS * MS                 # 3969
L = (MO - 1) * MS + MO + 2  # 3522 (even; last needed flat index is 63*55+55)
NCORES = 8
SPC = N // NCORES           # samples per core
BLK = 512
NBLK = (F + BLK - 1) // BLK   # 8
NBLK2 = (L + BLK - 1) // BLK  # 7
HALF = 4 * BLK                # y-piece size for x staging (2048)
AF = mybir.ActivationFunctionType
f32 = mybir.dt.float32
bf16 = mybir.dt.bfloat16

_CACHE = {}


def _build():
    nc = bacc.Bacc("TRN2", target_bir_lowering=False, debug=False)
    z_in = nc.declare_dram_parameter("z", [SPC, C, KS, KS], f32, isOutput=False)
    x_in = nc.declare_dram_parameter("x", [SPC, C, MS, MS], f32, isOutput=False)
    w_in = nc.declare_dram_parameter("w", [C], f32, isOutput=False)
    out = nc.declare_dram_parameter("out", [SPC, 1, MO, MO], f32, isOutput=True)
    FP = F + 8  # padded row pitch in the DRAM scratch (AP slack)
    pl_dram = nc.dram_tensor("pl_scratch", [SPC, 64, FP], bf16)

    xflat = x_in.rearrange("s (k c) h w -> s k c (h w)", c=128)  # [SPC, 2, 128, F]

    with tile.TileContext(nc) as tc:
        with (
            tc.tile_pool(name="xstage", bufs=5) as xstage,
            tc.tile_pool(name="sxq", bufs=8) as sxq,
            tc.tile_pool(name="zpool", bufs=2) as zpool,
            tc.tile_pool(name="plane", bufs=4) as plane,
            tc.tile_pool(name="gath", bufs=2) as gath,
            tc.tile_pool(name="opool", bufs=2) as opool,
            tc.tile_pool(name="psum", bufs=4, space="PSUM") as psum,
            tc.tile_pool(name="psum2", bufs=1, space="PSUM") as psum2,
        ):
            # all-ones [128, 1] bf16 for the tap-reduction matmul
            ones = opool.tile([128, 1], bf16, name="ones")
            nc.gpsimd.memset(ones[:], 1.0)

            # ---- input DMAs up-front, all on the Sync HWDGE ring ----
            # (small z/w loads first so szw never gates the matmuls)
            wt = zpool.tile([128, 2], f32, name="wt")
            nc.sync.dma_start(wt[:], w_in.rearrange("(k c) -> c k", c=128))
            zts = []
            for s in range(SPC):
                zt = zpool.tile([128, 2, KS * KS], f32, tag="zt", name=f"zt{s}")
                nc.sync.dma_start(
                    zt[:], z_in[s].rearrange("(k c) p q -> c k (p q)", c=128)
                )
                zts.append(zt)
            w64 = zpool.tile([128, 2], f32, name="w64")
            nc.scalar.mul(w64[:], wt[:], 1.0 / (KS * KS))
            xst = {}
            for s in range(SPC):
                for h in range(2):
                    for k in range(2):
                        nh = min(HALF, F - h * HALF)
                        t = xstage.tile([128, HALF], f32, tag="xst",
                                        name=f"xst{s}{k}{h}")
                        nc.sync.dma_start(
                            t[:, :nh], xflat[s, k, :, h * HALF : h * HALF + nh]
                        )
                        xst[(s, k, h)] = t

            for s in range(SPC):
                # ---- z path: szw[c, k, t] = w[c]/64 * sqrt(z[c, t]) ----
                zsq = zpool.tile([128, 2, KS * KS], f32, tag="zsq", name=f"zsq{s}")
                szw = zpool.tile([128, 2, KS * KS], bf16, tag="szw", name=f"szw{s}")
                for k in range(2):
                    nc.scalar.activation(zsq[:, k, :], zts[s][:, k, :], AF.Sqrt)
                    nc.vector.tensor_scalar_mul(
                        szw[:, k, :], zsq[:, k, :], w64[:, k : k + 1]
                    )

                # ---- x path: per-piece sqrt (bf16), separate tiles so
                # matmuls only depend on the pieces they read ----
                sxp = {}
                for h in range(2):
                    for k in range(2):
                        nh = min(HALF, F - h * HALF)
                        t = sxq.tile([128, HALF], bf16, tag="sxp",
                                     name=f"sxp{s}{k}{h}")
                        nc.scalar.activation(
                            t[:, :nh], xst[(s, k, h)][:, :nh], AF.Sqrt
                        )
                        sxp[(k, h)] = t

                # ---- matmuls: plane[t, y] = sum_c szw[c, t] sx[c, y] ----
                plh = {}
                for h in range(2):
                    nhalf = min(HALF, F - h * HALF)
                    pl = plane.tile([64, HALF], bf16, tag="pl", name=f"pl{s}{h}")
                    plh[h] = pl
                    pss = [
                        psum.tile([64, BLK], f32, tag="ps", name=f"ps_{s}_{4*h+j}")
                        for j in range(4)
                    ]
                    for k in range(2):
                        for j in range(4):
                            b = 4 * h + j
                            nb = min(BLK, F - b * BLK)
                            nc.tensor.matmul(
                                pss[j][:, :nb],
                                szw[:, k, :],
                                sxp[(k, h)][:, j * BLK : j * BLK + nb],
                                start=(k == 0),
                                stop=(k == 1),
                            )
                    for j in range(4):
                        b = 4 * h + j
                        nb = min(BLK, F - b * BLK)
                        # evict (cast bf16) on DVE
                        nc.vector.tensor_copy(
                            pl[:, j * BLK : j * BLK + nb], pss[j][:, :nb]
                        )
                    # dump this half of the plane to DRAM scratch (HWDGE)
                    nc.sync.dma_start(
                        pl_dram[s, :, h * HALF : h * HALF + nhalf], pl[:, :nhalf]
                    )

            # stage-2, chunked so chunk A only needs the h0 dump:
            #   A: u in [0, 1536)  (3 blocks)   B: u in [1536, L)  (4 blocks)
            CH = [(0, 3 * BLK), (3 * BLK, L - 3 * BLK)]
            for s in range(SPC):
                obuf = opool.tile([1, 3584], f32, tag=f"ob{s}", name=f"obuf{s}")
                for ci, (u0, ulen) in enumerate(CH):
                    # gather with per-tap shift: A[t, u] = plane[t, u0+u+off(t)]
                    a2 = gath.tile([64, 2048], bf16, tag="a2", name=f"a2_{s}{ci}")
                    src = bass.AP(
                        pl_dram[:].tensor,
                        s * 64 * FP + u0,
                        [[8 * FP + MS, 8], [FP + 1, 8], [1, ulen]],
                    )
                    nc.gpsimd.dma_start(a2[:, :ulen], src)

                    # tap reduction: o[u] = sum_t A[t, u]
                    ps2 = psum2.tile([1, 2048], f32, tag="ps2", name=f"ps2_{s}{ci}")
                    for b in range((ulen + BLK - 1) // BLK):
                        nb = min(BLK, ulen - b * BLK)
                        nc.tensor.matmul(
                            ps2[:, b * BLK : b * BLK + nb],
                            ones[0:64, :],
                            a2[:, b * BLK : b * BLK + nb],
                            start=True,
                            stop=True,
                        )
                    if ci == 0:
                        nc.scalar.copy(obuf[0:1, u0 : u0 + ulen], ps2[:, :ulen])
                    else:
                        nc.vector.tensor_copy(
                            obuf[0:1, u0 : u0 + ulen], ps2[:, :ulen]
                        )

                # ---- extract valid rows: out[i, j] = o[63 i + j] ----
                osrc = obuf[0:1, 0 : MO * MS].rearrange("p (i j) -> p i j", i=MO)[
                    :, :, 0:MO
                ]
                nc.gpsimd.dma_start(out[s, 0].unsqueeze(0), osrc)

    nc.compile()
    return nc


def _get_nc():
    if "nc" not in _CACHE:
        _CACHE["nc"] = _build()
    return _CACHE["nc"]


def _run(z, x, weights, **runkw):
    z = np.ascontiguousarray(np.asarray(z), dtype=np.float32)
    x = np.ascontiguousarray(np.asarray(x), dtype=np.float32)
    w = np.ascontiguousarray(np.asarray(weights), dtype=np.float32).reshape(C)
    in_maps = []
    for i in range(NCORES):
        lo, hi = i * SPC, (i + 1) * SPC
        in_maps.append({"z": z[lo:hi], "x": x[lo:hi], "w": w})
    nc = _get_nc()
    res = run_bass_kernel_spmd(nc, in_maps, core_ids=list(range(NCORES)), **runkw)
    full = np.concatenate([res.results[i]["out"] for i in range(NCORES)], axis=0)
    return full, res


def kernel(z, x, weights):
    full, _ = _run(z, x, weights)
    return full
